# revision 8
# baseline (speedup 1.0000x reference)
"""Trainium2 Bass kernel for nn_Attention (sparse_attention variant).

Computes, for inputs hidden/encoder_outputs [B,S,D], c_t [B,D], W [OUT,3D],
b [OUT], v [OUT]:

    cat       = concat([hidden, broadcast(c_t), encoder_outputs], axis=2)
    energy    = relu(cat @ W.T + b)            # [B, S, OUT]
    attention = energy @ v                     # [B, S]
    out       = softmax(attention, axis=1)

Strategy (8 NeuronCores, data-parallel over batch, 2 batches/core):
  - Weights are layout-prepped on host (transposed + fp16 cast, as for a
    deployed model): wT [128,16,1024] holds W1^T|W3^T chunks, w2T the W2^T
    chunks.  Loaded with plain linear DMAs split across both HWDGE rings.
  - X = [hidden | enc] is pipelined fully on-chip per 128-row s-tile:
    natural fp32 chunk DMA (h on sync ring, e on scalar ring) -> DVE
    fp32->fp16 cast -> SBUF->SBUF xbar-transpose DMA into the [f, s]
    layout the PE needs.  No DRAM scratch round-trip, no SWDGE.
  - Main loop per s-tile: accumulate pre[s,o] over 16 f-chunks x 2 PSUM
    banks; VectorE adds the per-batch c2 row (c_t@W2.T + b, computed on
    PE after the first s-tile) and does fused relu*v + row-sum into the
    attention logits.
  - Softmax over S=2048 per batch: DVE free-dim reduce + GpSimd partition
    all-reduce, ScalarE exp, DVE normalize, store on the gpsimd ring.
"""

import sys
import numpy as np

for _p in ("/opt/trn_rl_repo",):
    if _p not in sys.path:
        sys.path.insert(0, _p)

import concourse.bass as bass
import concourse.bacc as bacc
import concourse.tile as tile
from concourse import mybir, bass_isa
from concourse.bass_utils import run_bass_kernel_spmd

F32 = mybir.dt.float32
F16 = mybir.dt.float16
BF16 = mybir.dt.bfloat16
AF = mybir.ActivationFunctionType
ALU = mybir.AluOpType

B, S, D, OUT = 16, 2048, 1024, 1024
N_CORES = 8
B_LOC = B // N_CORES            # batches per core
S_LOC = B_LOC * S               # 4096 rows of X per core
N_ST = S_LOC // 128             # 32 s-tiles per core
ST_PER_B = S // 128             # 16 s-tiles per batch
FC = D // 128                   # 8 feature chunks per tensor
NB = OUT // 512                 # 2 PSUM banks across OUT
PREF = 4                        # s-tile prefetch depth


def build_nc():
    nc = bacc.Bacc("TRN2", target_bir_lowering=False, debug=False,
                   num_devices=N_CORES, dynamic_dma_scratch_size=32768)

    hid = nc.dram_tensor("hidden", [S_LOC, D], F32, kind="ExternalInput").ap()
    enc = nc.dram_tensor("enc", [S_LOC, D], F32, kind="ExternalInput").ap()
    ct = nc.dram_tensor("ct", [B_LOC, D], F32, kind="ExternalInput").ap()
    wTd = nc.dram_tensor("wT", [128, 2 * FC, OUT], F16,
                         kind="ExternalInput").ap()
    w2Td = nc.dram_tensor("w2T", [128, FC, OUT], F16,
                          kind="ExternalInput").ap()
    bd = nc.dram_tensor("b", [OUT], F16, kind="ExternalInput").ap()
    vd = nc.dram_tensor("v", [OUT], F16, kind="ExternalInput").ap()
    outd = nc.dram_tensor("out", [B_LOC, S], F32, kind="ExternalOutput").ap()

    with tile.TileContext(nc) as tc:
        with (
            tc.tile_pool(name="const", bufs=1) as cpool,
            tc.tile_pool(name="wT", bufs=1) as wpool,
            tc.tile_pool(name="xn32", bufs=3) as xn32,
            tc.tile_pool(name="xn16", bufs=3) as xn16,
            tc.tile_pool(name="xT", bufs=PREF) as xTp,
            tc.tile_pool(name="scratch", bufs=2) as spool,
            tc.tile_pool(name="sm", bufs=2) as smpool,
            tc.tile_pool(name="eps", bufs=3, space=bass.MemorySpace.PSUM) as eps,
            tc.tile_pool(name="pps", bufs=1, space=bass.MemorySpace.PSUM) as pps,
        ):
            # ---- small constants ------------------------------------------
            ones_k1 = cpool.tile([1, 128], F16)
            nc.vector.memset(ones_k1[:], 1.0)
            att_all = cpool.tile([128, N_ST], F32)   # attention logits

            ctT_f = cpool.tile([128, FC, B_LOC], F32)
            for bb in range(B_LOC):
                nc.sync.dma_start(ctT_f[:, :, bb],
                                  ct[bb].rearrange("(fc p) -> p fc", p=128))
            b_h = cpool.tile([1, OUT], F16)
            nc.sync.dma_start(b_h[:], bd[None, :])
            v_h = cpool.tile([1, OUT], F16)
            nc.sync.dma_start(v_h[:], vd[None, :])
            ctT_h = cpool.tile([128, FC, B_LOC], F16)
            nc.vector.tensor_copy(ctT_h[:], ctT_f[:])

            # ---- weights: halves split across both rings ------------------
            wT = wpool.tile([128, 2 * FC, OUT], F16)
            nc.sync.dma_start(wT[:, :FC, :], wTd[:, :FC, :])
            nc.scalar.dma_start(wT[:, FC:, :], wTd[:, FC:, :])
            w2T = wpool.tile([128, FC, OUT], F16)
            nc.scalar.dma_start(w2T[:], w2Td)

            # ---- X pipeline: load fp32 / DVE cast / xbar transpose --------
            def emit_xchunk(st):
                rows = slice(st * 128, (st + 1) * 128)
                nh = xn32.tile([128, D], F32, tag="nh")
                nc.sync.dma_start(nh[:], hid[rows, :])
                ne = xn32.tile([128, D], F32, tag="ne")
                nc.scalar.dma_start(ne[:], enc[rows, :])
                nh16 = xn16.tile([128, D], F16, tag="nh")
                nc.vector.tensor_copy(nh16[:], nh[:])
                ne16 = xn16.tile([128, D], F16, tag="ne")
                nc.vector.tensor_copy(ne16[:], ne[:])
                th = xTp.tile([128, FC, 128], F16, tag="th")
                nc.sync.dma_start(th[:], nh16[:], transpose=True)
                te = xTp.tile([128, FC, 128], F16, tag="te")
                nc.scalar.dma_start(te[:], ne16[:], transpose=True)
                return th, te

            c2bc_sb = []

            def emit_c2_vbc():
                # c2[b,:] = c_t[b] @ W2.T + b, broadcast to 128 rows
                for bb in range(B_LOC):
                    c2_ps = pps.tile([1, OUT], F32, tag="pp")
                    for ob in range(NB):
                        sl = slice(ob * 512, (ob + 1) * 512)
                        for fc in range(FC):
                            nc.tensor.matmul(c2_ps[:, sl],
                                             ctT_h[:, fc, bb:bb + 1],
                                             w2T[:, fc, sl],
                                             start=(fc == 0), stop=False)
                        nc.tensor.matmul(c2_ps[:, sl], ones_k1[:, :1],
                                         b_h[:, sl], start=False, stop=True)
                    c2b = cpool.tile([1, OUT], F16, tag=f"c2_{bb}")
                    nc.vector.tensor_copy(c2b[:], c2_ps[:])
                    c2bc_ps = pps.tile([128, OUT], F32, tag="pp")
                    for ob in range(NB):
                        sl = slice(ob * 512, (ob + 1) * 512)
                        nc.tensor.matmul(c2bc_ps[:, sl], ones_k1[:],
                                         c2b[:, sl], start=True, stop=True)
                    c2bc = cpool.tile([128, OUT], F16, tag=f"c2bc_{bb}")
                    nc.vector.tensor_copy(c2bc[:], c2bc_ps[:])
                    c2bc_sb.append(c2bc)
                # vbc[p, o] = v[o] (fp16) for the fused relu*v epilogue
                vbc_ps = pps.tile([128, OUT], F32, tag="pp")
                for ob in range(NB):
                    sl = slice(ob * 512, (ob + 1) * 512)
                    nc.tensor.matmul(vbc_ps[:, sl], ones_k1[:], v_h[:, sl],
                                     start=True, stop=True)
                vbc = cpool.tile([128, OUT], F16)
                nc.vector.tensor_copy(vbc[:], vbc_ps[:])
                return vbc

            def emit_softmax(bb):
                sl = slice(bb * ST_PER_B, (bb + 1) * ST_PER_B)
                m1 = smpool.tile([128, 1], F32, tag="m1")
                nc.vector.tensor_reduce(m1[:], att_all[:, sl],
                                        axis=mybir.AxisListType.X,
                                        op=ALU.max)
                mall = smpool.tile([128, 1], F32, tag="mall")
                nc.gpsimd.partition_all_reduce(mall[:], m1[:], channels=128,
                                               reduce_op=bass_isa.ReduceOp.max)
                nmall = smpool.tile([128, 1], F32, tag="nmall")
                nc.vector.tensor_scalar_mul(nmall[:], mall[:], -1.0)
                ex = smpool.tile([128, ST_PER_B], F32, tag="ex")
                rs = smpool.tile([128, 1], F32, tag="rs")
                nc.scalar.activation(ex[:], att_all[:, sl], AF.Exp,
                                     bias=nmall[:], accum_out=rs[:])
                tot = smpool.tile([128, 1], F32, tag="tot")
                nc.gpsimd.partition_all_reduce(tot[:], rs[:], channels=128,
                                               reduce_op=bass_isa.ReduceOp.add)
                rec = smpool.tile([128, 1], F32, tag="rec")
                nc.vector.reciprocal(rec[:], tot[:])
                res_t = smpool.tile([128, ST_PER_B], F32, tag="res")
                nc.vector.tensor_scalar_mul(res_t[:], ex[:], rec[:])
                nc.gpsimd.dma_start(
                    outd[bb].rearrange("(stl p) -> p stl", p=128), res_t[:])

            # ---- main loop ------------------------------------------------
            tiles = [emit_xchunk(st) for st in range(PREF)]
            vbc = None
            for st in range(N_ST):
                b_idx = st // ST_PER_B
                th, te = tiles[st]

                e_ps = eps.tile([128, OUT], F32, tag="eps")
                for ob in range(NB):
                    sl = slice(ob * 512, (ob + 1) * 512)
                    for fc in range(FC):
                        nc.tensor.matmul(e_ps[:, sl], th[:, fc, :],
                                         wT[:, fc, sl],
                                         start=(fc == 0), stop=False)
                    for fc in range(FC):
                        nc.tensor.matmul(e_ps[:, sl], te[:, fc, :],
                                         wT[:, FC + fc, sl],
                                         start=False, stop=(fc == FC - 1))

                if st == 0:
                    vbc = emit_c2_vbc()
                if st + PREF < N_ST:
                    tiles.append(emit_xchunk(st + PREF))

                # pre += c2[b] (broadcast), then
                # att[st] = sum_o relu(pre) * v  (fused on VectorE)
                nc.vector.tensor_add(e_ps[:], e_ps[:], c2bc_sb[b_idx][:])
                relu_out = spool.tile([128, OUT], BF16, tag="relu")
                nc.vector.scalar_tensor_tensor(
                    relu_out[:], e_ps[:], 0.0, vbc[:],
                    op0=ALU.max, op1=ALU.mult,
                    accum_out=att_all[:, st:st + 1])
                if st % ST_PER_B == ST_PER_B - 1:
                    emit_softmax(st // ST_PER_B)

    nc.compile()
    return nc


_NC = None


def _get_nc():
    global _NC
    if _NC is None:
        _NC = build_nc()
    return _NC


def _prep_weights(W, b, v):
    W = np.ascontiguousarray(W, dtype=np.float32)
    # wT[p, j, o] = W1[o, j*128+p] for j<8, W3[o, (j-8)*128+p] for j>=8
    W13T = np.concatenate([W[:, :D].T, W[:, 2 * D:].T], axis=0)  # [2D, OUT]
    wT = np.ascontiguousarray(
        W13T.reshape(2 * FC, 128, OUT).transpose(1, 0, 2).astype(np.float16))
    w2T = np.ascontiguousarray(
        W[:, D:2 * D].T.reshape(FC, 128, OUT).transpose(1, 0, 2)
        .astype(np.float16))
    b = np.ascontiguousarray(b, dtype=np.float16)
    v = np.ascontiguousarray(v, dtype=np.float16)
    return wT, w2T, b, v


def _in_maps(hidden, encoder_outputs, c_t, W, b, v):
    hidden = np.ascontiguousarray(hidden, dtype=np.float32)
    encoder_outputs = np.ascontiguousarray(encoder_outputs, dtype=np.float32)
    c_t = np.ascontiguousarray(c_t, dtype=np.float32)
    wT, w2T, b, v = _prep_weights(W, b, v)
    maps = []
    for i in range(N_CORES):
        bs = slice(i * B_LOC, (i + 1) * B_LOC)
        maps.append({
            "hidden": hidden[bs].reshape(S_LOC, D),
            "enc": encoder_outputs[bs].reshape(S_LOC, D),
            "ct": c_t[bs],
            "wT": wT, "w2T": w2T, "b": b, "v": v,
        })
    return maps


def run(hidden, encoder_outputs, c_t, W, b, v, trace=False, tmpdir=None):
    nc = _get_nc()
    maps = _in_maps(hidden, encoder_outputs, c_t, W, b, v)
    res = run_bass_kernel_spmd(nc, maps, list(range(N_CORES)), trace=trace,
                               tmpdir=tmpdir)
    out = np.concatenate([res.results[i]["out"] for i in range(N_CORES)],
                         axis=0)
    return out, res


def kernel(hidden, encoder_outputs, c_t, W, b, v):
    out, _ = run(hidden, encoder_outputs, c_t, W, b, v)
    return out
